# revision 3
# baseline (speedup 1.0000x reference)
"""CenterLoss kernel for 8 TRN2 NeuronCores.

Math: the reference builds the full [B, C] squared-distance matrix, masks it
to the true-label column, clamps elementwise to [1e-12, 1e12] and sums.
Every masked-out entry contributes exactly CLAMP_MIN, so

    loss = ( sum_i clip(||x_i - c_{l_i}||^2, 1e-12, 1e12)
             + B*(C-1)*1e-12 ) / B

Only the B gathered center rows ever matter.  Sharding strategy: shard the
batch (B=1024 -> 128 rows per core); the gather of centers[labels] happens
while building the per-core input shards.  Each core computes clipped
per-row squared distances [128, 1]; the host sums the 8 partials and adds
the analytic clamp-floor term.
"""

import os
import sys

import numpy as np

for _p in ("/opt/trn_rl_repo",):
    if os.path.isdir(_p) and _p not in sys.path:
        sys.path.insert(0, _p)

import concourse.bacc as bacc
import concourse.mybir as mybir
import concourse.tile as tile
from concourse.bass_utils import run_bass_kernel_spmd

B, C, D = 1024, 100000, 256
N_CORES = 8
ROWS = B // N_CORES  # 128 rows per core == SBUF partition count
CLAMP_MIN, CLAMP_MAX = 1e-12, 1e12

_cached_nc = None


def _build():
    """Per-core kernel: out[i] = clip(sum_d (x[i,d]-c[i,d])^2, 1e-12, 1e12)."""
    nc = bacc.Bacc(
        "TRN2",
        target_bir_lowering=False,
        debug=False,
        enable_asserts=False,
        num_devices=N_CORES,
    )
    x_d = nc.dram_tensor("x", [ROWS, D], mybir.dt.float32, kind="ExternalInput")
    c_d = nc.dram_tensor("c", [ROWS, D], mybir.dt.float32, kind="ExternalInput")
    out_d = nc.dram_tensor("out", [ROWS, 1], mybir.dt.float32, kind="ExternalOutput")

    with tile.TileContext(nc) as tc:
        with tc.tile_pool(name="sbuf", bufs=1) as pool:
            xt = pool.tile([ROWS, D], mybir.dt.float32)
            ct = pool.tile([ROWS, D], mybir.dt.float32)
            nc.sync.dma_start(xt[:], x_d[:])
            nc.sync.dma_start(ct[:], c_d[:])

            diff = pool.tile([ROWS, D], mybir.dt.float32)
            nc.vector.tensor_sub(diff[:], xt[:], ct[:])

            sq = pool.tile([ROWS, D], mybir.dt.float32)
            dist = pool.tile([ROWS, 1], mybir.dt.float32)
            nc.vector.tensor_mul(sq[:], diff[:], diff[:])
            nc.vector.reduce_sum(dist[:], sq[:], axis=mybir.AxisListType.X)
            # dist = min(max(dist, CLAMP_MIN), CLAMP_MAX)
            nc.vector.tensor_scalar(
                out=dist[:],
                in0=dist[:],
                scalar1=CLAMP_MIN,
                scalar2=CLAMP_MAX,
                op0=mybir.AluOpType.max,
                op1=mybir.AluOpType.min,
            )
            nc.sync.dma_start(out_d[:], dist[:])

    nc.compile()
    return nc


def _make_in_maps(x, labels, centers):
    x = np.ascontiguousarray(np.asarray(x, dtype=np.float32))
    centers = np.asarray(centers, dtype=np.float32)
    labels = np.asarray(labels)
    # Shard: gather each row's true center while building per-core inputs.
    gathered = np.ascontiguousarray(centers[labels])  # [B, D] f32
    return [
        {
            "x": x[k * ROWS : (k + 1) * ROWS],
            "c": gathered[k * ROWS : (k + 1) * ROWS],
        }
        for k in range(N_CORES)
    ]


def kernel(x, labels, centers):
    global _cached_nc
    if _cached_nc is None:
        _cached_nc = _build()
    nc = _cached_nc

    in_maps = _make_in_maps(x, labels, centers)
    res = run_bass_kernel_spmd(nc, in_maps, core_ids=list(range(N_CORES)))

    dists = np.concatenate([r["out"].reshape(-1) for r in res.results])
    total = dists.sum(dtype=np.float64) + B * (C - 1) * CLAMP_MIN
    return np.float32(total / B)


# revision 4
# speedup vs baseline: 1.5540x; 1.5540x over previous
"""CenterLoss kernel for 8 TRN2 NeuronCores (Bass, raw bacc).

Math: the reference builds the full [B, C] squared-distance matrix, masks it
to the true-label column, clamps elementwise to [1e-12, 1e12] and sums:

    distmat[i, j] = ||x_i||^2 + ||c_j||^2 - 2 x_i . c_j
    loss = sum(clip(distmat * onehot(labels), 1e-12, 1e12)) / B

Every masked-out entry contributes exactly CLAMP_MIN, so

    loss = ( sum_i clip(||x_i - c_{l_i}||^2, 1e-12, 1e12)
             + B*(C-1)*1e-12 ) / B

Only the B gathered center rows matter.  Sharding: the batch is split over
the 8 cores (128 rows each); building a core's shard gathers its rows'
true centers from the centers table.  Per core the device kernel:
  - DMAs [128, 512] (x_rows | gathered_center_rows) into SBUF
    (split across the SP and Activation HWDGE queues),
  - DVE: diff = x - c; sq = diff*diff; dist = rowsum(sq);
    dist = clip(dist, 1e-12, 1e12),
  - PE: matmul with a ones vector reduces the 128 clipped row distances
    to a single [1,1] scalar (a 4-byte output => one DMA packet instead
    of 128 scattered ones),
  - DMAs the scalar out.
The host sums the 8 partial scalars, adds the analytic clamp-floor term
B*(C-1)*1e-12, and divides by B.

Raw-bacc (no TileContext) discipline, learned the hard way:
  - Semaphores persist across NEFF executions on a core; with
    target_bir_lowering=False nothing clears them, so a prior run's
    leftovers satisfy waits instantly and engines race ahead of DMAs.
    The kernel clears its own semaphore range up front, bracketed by
    all-engine barriers.
  - The DVE is deeply pipelined and has no implicit RAW interlock between
    instructions; every dependent DVE->DVE pair and every cross-engine
    publish needs an explicit drain() first (Tile normally inserts these).
"""

import os
import sys

import numpy as np

for _p in ("/opt/trn_rl_repo",):
    if os.path.isdir(_p) and _p not in sys.path:
        sys.path.insert(0, _p)

import concourse.bacc as bacc
import concourse.mybir as mybir
from concourse.bass_utils import run_bass_kernel_spmd

B, C, D = 1024, 100000, 256
N_CORES = 8
ROWS = B // N_CORES  # 128 rows per core == SBUF partition count
CLAMP_MIN, CLAMP_MAX = 1e-12, 1e12
F32 = mybir.dt.float32

_cached_nc = None


def _build():
    nc = bacc.Bacc(
        "TRN2",
        target_bir_lowering=False,
        debug=False,
        enable_asserts=False,
        num_devices=N_CORES,
    )
    xc_d = nc.dram_tensor("xc", [ROWS, 2 * D], F32, kind="ExternalInput")
    out_d = nc.dram_tensor("out", [1, 1], F32, kind="ExternalOutput")
    ones = nc.const_aps.tensor(1.0, [ROWS, 1], F32)
    with (
        nc.sbuf_tensor([ROWS, 2 * D], F32) as t,
        nc.sbuf_tensor([ROWS, D], F32) as diff,
        nc.sbuf_tensor([ROWS, D], F32) as sq,
        nc.sbuf_tensor([ROWS, 1], F32) as dist,
        nc.sbuf_tensor([1, 1], F32) as res_sb,
        nc.psum_tensor([1, 1], F32) as ps,
        nc.semaphore() as in_sem,
        nc.semaphore() as v_sem,
        nc.semaphore() as pe_sem,
        nc.semaphore() as out_sem,
    ):
        # Semaphore hygiene (see module docstring).
        sems = [in_sem, v_sem, pe_sem, out_sem]
        lo = min(s.num for s in sems)
        hi = max(s.num for s in sems) + 1
        nc.all_engine_barrier()
        nc.gpsimd.dma_reset(range(lo, hi))
        nc.gpsimd.sem_clear(range(lo, hi))
        nc.all_engine_barrier()

        # Input: x rows in cols [0,D), gathered center rows in cols [D,2D).
        nc.sync.dma_start(t[:, 0:D], xc_d[:, 0:D]).then_inc(in_sem, 16)
        nc.scalar.dma_start(t[:, D : 2 * D], xc_d[:, D : 2 * D]).then_inc(in_sem, 16)

        nc.vector.wait_ge(in_sem, 32)
        nc.vector.tensor_sub(diff[:], t[:, 0:D], t[:, D : 2 * D])
        nc.vector.drain()
        nc.vector.tensor_mul(sq[:], diff[:], diff[:])
        nc.vector.drain()
        nc.vector.reduce_sum(dist[:], sq[:], axis=mybir.AxisListType.X)
        nc.vector.drain()
        nc.vector.tensor_scalar(
            out=dist[:], in0=dist[:], scalar1=CLAMP_MIN, scalar2=CLAMP_MAX,
            op0=mybir.AluOpType.max, op1=mybir.AluOpType.min,
        )
        nc.vector.drain().then_inc(v_sem, 1)

        # Partition-reduce: ones.T @ dist -> [1,1] in PSUM.
        nc.tensor.wait_ge(v_sem, 1)
        nc.tensor.matmul(ps[:], dist[:], ones)
        nc.tensor.drain().then_inc(pe_sem, 1)

        nc.vector.wait_ge(pe_sem, 1)
        nc.vector.tensor_copy(res_sb[:], ps[:])
        nc.vector.drain().then_inc(v_sem, 1)

        nc.sync.wait_ge(v_sem, 2)
        nc.sync.dma_start(out_d[:], res_sb[:]).then_inc(out_sem, 16)
        nc.sync.wait_ge(out_sem, 16)
    nc.compile()
    return nc


def _make_in_maps(x, labels, centers):
    x = np.asarray(x, dtype=np.float32)
    centers = np.asarray(centers, dtype=np.float32)
    labels = np.asarray(labels)
    xc = np.concatenate([x, centers[labels]], axis=1)  # [B, 2D]
    xc = np.ascontiguousarray(xc)
    return [{"xc": xc[k * ROWS : (k + 1) * ROWS]} for k in range(N_CORES)]


def kernel(x, labels, centers):
    global _cached_nc
    if _cached_nc is None:
        _cached_nc = _build()
    nc = _cached_nc

    in_maps = _make_in_maps(x, labels, centers)
    res = run_bass_kernel_spmd(nc, in_maps, core_ids=list(range(N_CORES)))

    partial = sum(float(r["out"][0, 0]) for r in res.results)
    total = partial + B * (C - 1) * CLAMP_MIN
    return np.float32(total / B)
